# revision 2
# baseline (speedup 1.0000x reference)
"""Trainium2 Bass kernel: NoiseEstimation (Sobel magnitude G, orientation
coherence C, 5x5 local variance V) over (16,1,512,512) fp32 input.

Math (trig-free):
  gx, gy = Sobel 3x3 cross-correlation, zero pad 1
  g2 = gx^2 + gy^2 (bf16);  r = abs_rsqrt(g2);  G = g2 * r
  u = gx*r, v = gy*r  (cos/sin of theta, bf16)
  C = u*box3c(u) + v*box3c(v)      box3c = (3x3 sum - center)/8, replicate
                                   pad; the -1/8 folds in via u^2+v^2=1
  V = box5(x^2)/25 - (box5(x)/25)^2     box5 = 5x5 sum, zero pad

Distribution: pure data parallel, 2 images per core on 8 NeuronCores.
Per core the 2 images are stacked into a [1024, 512] row space processed
as 9 overlapping 128-row tiles, software-pipelined across engines.

Engine assignment (balanced under the v2 timeline cost model; GPSIMD
cannot touch PSUM on hardware and its add/mult runs at 0.42 efficiency,
so Pool gets only cheap SBUF work):
  Tensor : vertical stencil taps as banded-matrix matmuls (15/tile):
           sobel (5, fp32r, gy-first so its Square overlaps gx), box5
           verticals p5/q5 (bf16), -I @ mu^2 folded into the open q5
           accumulation group (V = q5 - mu^2), su/sv box3 on the unit
           gradients (6, bf16, horizontal taps via shifted rhs slices)
  Act    : xx = x^2 (bf16), pq2 = Square(gxy) [2x512] PSUM->SBUF,
           rinv = Abs_rsqrt(g2), m2 = Square(p5), V = Copy(q5) -> fp16
           (all four funcs live in one act table set - no reloads)
  DVE    : hx/hxx = horizontal 5-window running sums (tensor_tensor_scan,
           state = (x[t+5]+state)-x[t]), uv = gxy*rinv (PSUM mixed, the
           only engine that can), uv replicate-pad edge copies,
           t12 = uv*suv (PSUM mixed), C = t1+t2 -> fp16 (bf16 2x)
  Pool   : g2 = p2+q2, G = g2*rinv -> fp16, x pad memsets
  SP/Act : stores ride SP, input prefetches ride the Act DGE queue so
           they never queue behind stores (loads have no waits -> no
           head-of-line blocking); the LAST store issues on the Act
           queue - the final SP-queue DMA is not reliably drained at
           program end (silent tail corruption otherwise)

Output is staged in fp16 (halves store traffic vs fp32; host upcasts,
well inside the 2e-2 tolerance). PSUM: gxy[2]x2 double-buffered, p5[1],
q5[1], suv[2] = 8 banks. PE warmup burst rides out the cold p-states.
TimelineSim (v2 cost model): ~51.2us/core vs 87.2us for the previous
Pool-heavy kernel; CoreSim (v1): ~50.5us. Verified on the 8 axon trn2
cores: rel err 4.37e-3 (gate 2e-2).
"""

import numpy as np
from contextlib import ExitStack

import concourse.bass as bass
import concourse.bacc as bacc
import concourse.tile as tile
import concourse.mybir as mybir
from concourse import bass_utils

F32 = mybir.dt.float32
F32R = mybir.dt.float32r
F16 = mybir.dt.float16
BF16 = mybir.dt.bfloat16
AL = mybir.AluOpType
AF = mybir.ActivationFunctionType

H = 512
W = 512
N_CORES = 8
IPC = 2                    # images per core
ROWS = IPC * H             # 1024 stacked rows per core
TILE_OFS = [0, 124, 248, 372, 496, 620, 744, 868, 896]
N_TILES = len(TILE_OFS)
# tile -> band-matrix set (0=t0 top, 1=interior, 2=image boundary, 3=bottom)
TILE_SET = [0, 1, 1, 1, 2, 1, 1, 1, 3]
SET_OFS = [0, 124, 496, 896]
N_SETS = 4
NF_MATS = 4   # fp32 per set: V121p, V121n, Vd1, Vd2
NB_MATS = 3   # bf16 per set: B3, B3c, B5


def _valid_range(ti):
    """Valid output partition range [m0, m1) for tile ti."""
    if ti == 0:
        return 0, 126
    if ti == N_TILES - 1:
        start_g = TILE_OFS[ti - 1] + 126   # first row not covered by prev tile
        return start_g - TILE_OFS[ti], 128
    return 2, 126


def _build_mat_set(ofs):
    """Banded 128x128 lhsT matrices for a tile at row offset `ofs`.

    lhsT[k, m] = weight of tile input row k contributing to output row m.
    Image edges (zero pad for sobel/box5, replicate for box3) are encoded
    per-row; tiles spanning the two-image boundary get block-diagonal bands.
    """
    fmats = [np.zeros((128, 128), np.float32) for _ in range(NF_MATS)]
    bmats = [np.zeros((128, 128), np.float32) for _ in range(NB_MATS)]
    V121p, V121n, Vd1, Vd2 = fmats
    B3, B3c, B5 = bmats
    for m in range(128):
        g = ofs + m
        img = g // H
        if img >= IPC:
            continue
        lo, hi = img * H, img * H + H - 1

        def add(mat, d, w, rep=False):
            gt = g + d
            if gt < lo or gt > hi:
                if not rep:
                    return
                gt = min(max(gt, lo), hi)
            k = gt - ofs
            if 0 <= k < 128:
                mat[k, m] += w

        for d, wgt in ((-1, 1.0), (0, 2.0), (1, 1.0)):
            add(V121p, d, wgt)
            add(V121n, d, -wgt)
        for d, wgt in ((-1, -1.0), (1, 1.0)):
            add(Vd1, d, wgt)
            add(Vd2, d, 2.0 * wgt)
        for d in (-2, -1, 0, 1, 2):
            add(B5, d, 0.04)
        for d in (-1, 0, 1):
            add(B3, d, 0.125, rep=True)
            add(B3c, d, 0.125, rep=True)
        B3c[m, m] -= 0.125  # folds C's "- 1/8" via u^2 + v^2 = 1
    return fmats, bmats


def _mats_const():
    """fp32 sobel mats [128, 4*4*128]; bf16 mats [128, (4*3+1)*128]
    (B3, B3c, B5 per set + trailing -I)."""
    arrf = np.zeros((128, N_SETS * NF_MATS * 128), np.float32)
    arrb = np.zeros((128, (N_SETS * NB_MATS + 1) * 128), np.float32)
    for s, ofs in enumerate(SET_OFS):
        fmats, bmats = _build_mat_set(ofs)
        for f, mat in enumerate(fmats):
            b = s * NF_MATS + f
            arrf[:, b * 128:(b + 1) * 128] = mat
        for f, mat in enumerate(bmats):
            b = s * NB_MATS + f
            arrb[:, b * 128:(b + 1) * 128] = mat
    arrb[:, N_SETS * NB_MATS * 128:] = -np.eye(128, dtype=np.float32)
    return arrf, arrb


def _emit(ctx: ExitStack, tc: "tile.TileContext", x_d, o_d, matsf_d, matsb_d):
    nc = tc.nc
    mpool = ctx.enter_context(tc.tile_pool(name="mats", bufs=1))
    xpool = ctx.enter_context(tc.tile_pool(name="xp", bufs=5))
    spool = ctx.enter_context(tc.tile_pool(name="sp", bufs=3))
    upool = ctx.enter_context(tc.tile_pool(name="up", bufs=3))
    opool = ctx.enter_context(tc.tile_pool(name="op", bufs=4))
    psA = ctx.enter_context(tc.tile_pool(name="psA", bufs=2, space="PSUM"))
    psP = ctx.enter_context(tc.tile_pool(name="psP", bufs=1, space="PSUM"))
    psQ = ctx.enter_context(tc.tile_pool(name="psQ", bufs=1, space="PSUM"))
    psS = ctx.enter_context(tc.tile_pool(name="psS", bufs=1, space="PSUM"))

    matsf_sb = mpool.tile([128, N_SETS * NF_MATS * 128], F32R, tag="matsf")
    matsb_sb = mpool.tile([128, (N_SETS * NB_MATS + 1) * 128], BF16, tag="matsb")

    def MF(s, f):
        b = s * NF_MATS + f
        return matsf_sb[:, b * 128:(b + 1) * 128]

    def MB(s, f):
        b = s * NB_MATS + f
        return matsb_sb[:, b * 128:(b + 1) * 128]

    In_m = matsb_sb[:, N_SETS * NB_MATS * 128:]

    # force the single act table set (abs_reciprocal_sqrt_and_small:
    # abs_rsqrt / square / copy)
    scratch = mpool.tile([128, 8], F32, tag="scr")
    nc.gpsimd.memset(scratch[:], 1.0)
    nc.scalar.activation(scratch[:, 0:4], scratch[:, 4:8],
                         AF.Abs_reciprocal_sqrt, bias=1e-35)

    # PE warmup: the PE clock ramps only while busy (HAM gate); a burst of
    # dummy N=128 matmuls on zeroed weights from t~0 rides out the cold
    # p-states during the initial DMA window so tile 0 runs at full rate.
    wmat = mpool.tile([128, 128], BF16, tag="wm")
    nc.gpsimd.memset(wmat[:], 0.0)
    wps = psP.tile([128, 512], F32, tag="p5")
    NWARM = 20
    for k in range(NWARM):
        nc.tensor.matmul(wps[:, 0:128], wmat[:], wmat[:],
                         start=(k == 0), stop=(k == NWARM - 1))

    # per-tile state carried across the software pipeline
    st = [None] * N_TILES
    xq = [None] * N_TILES
    sq = [None] * N_TILES

    def load_x(i):
        # x tile: [4 zero | 512 | 4 zero] cols; issued two slots ahead.
        # The pad columns are memset once per physical buffer (first 4
        # tiles): the DMA only ever writes cols 4:516, so they stay zero.
        x_t = xpool.tile([128, 520], F32R, tag="x")
        nc.scalar.dma_start(x_t[:, 4:516], x_d[TILE_OFS[i]:TILE_OFS[i] + 128, :])
        nc.gpsimd.memset(x_t[:, 0:4].bitcast(F32), 0.0)
        nc.gpsimd.memset(x_t[:, 516:520].bitcast(F32), 0.0)
        xq[i] = x_t

    def prep(i):
        # xx = x^2 (Act, bf16; zero pads square to zero), then the two
        # horizontal 5-window running sums as DVE scans:
        # state = (d0[t] + state) - d1[t]  ->  out[t] = sum x[t+1..t+5]
        x_t = xq[i]
        xf = x_t.bitcast(F32)
        xx_t = spool.tile([128, 518], BF16, tag="xx")
        nc.scalar.activation(xx_t[:], xf[:, 0:518], AF.Square)
        hx_t = spool.tile([128, 513], BF16, tag="hx")
        nc.vector.tensor_tensor_scan(hx_t[:], x_t[:, 5:518], x_t[:, 0:513],
                                     x_t[:, 4:5], AL.add, AL.subtract)
        hxx_t = spool.tile([128, 513], BF16, tag="hxx")
        nc.vector.tensor_tensor_scan(hxx_t[:], xx_t[:, 5:518], xx_t[:, 0:513],
                                     xx_t[:, 4:5], AL.add, AL.subtract)
        sq[i] = (hx_t, hxx_t)

    def front_mm(i):
        # PE-only: no upstream deps beyond x/hx/hxx -> issue first per slot.
        # gy first: its Square can overlap the gx matmuls, shortening the
        # rinv critical chain.
        s = TILE_SET[i]
        x_t = xq[i]

        def xr(j):
            return x_t[:, 4 + j:4 + j + 512]

        # ---- sobel via banded matmuls -> gxy PSUM [128, 2, 512] ----
        gxy = psA.tile([128, 2, 512], F32, tag="gxy")
        nc.tensor.matmul(gxy[:, 1, :], MF(s, 2), xr(-1), start=True, stop=False)
        nc.tensor.matmul(gxy[:, 1, :], MF(s, 2), xr(+1), start=False, stop=False)
        nc.tensor.matmul(gxy[:, 1, :], MF(s, 3), xr(0), start=False, stop=True)
        nc.tensor.matmul(gxy[:, 0, :], MF(s, 0), xr(+1), start=True, stop=False)
        nc.tensor.matmul(gxy[:, 0, :], MF(s, 1), xr(-1), start=False, stop=True)

        hx_t, hxx_t = sq[i]

        # ---- box5 sums: vertical band on the horizontal window sums ----
        p5 = psP.tile([128, 512], F32, tag="p5")
        nc.tensor.matmul(p5[:], MB(s, 2), hx_t[:, 1:513], start=True, stop=True)
        q5 = psQ.tile([128, 512], F32, tag="q5")
        nc.tensor.matmul(q5[:], MB(s, 2), hxx_t[:, 1:513], start=True, stop=False)

        st[i] = dict(s=s, gxy=gxy, p5=p5, q5=q5)

    def front_elem(i):
        # G-chain head + V; rinv lands at slot end, its consumers (uv, G)
        # run next slot so the spill is absorbed.
        d = st[i]
        pq2 = spool.tile([128, 2, 512], BF16, tag="pq2")
        nc.scalar.activation(pq2[:], d["gxy"][:], AF.Square)
        g2 = spool.tile([128, 512], BF16, tag="g2")
        nc.gpsimd.tensor_add(g2[:], pq2[:, 0, :], pq2[:, 1, :])
        m2 = spool.tile([128, 512], BF16, tag="m2")
        nc.scalar.activation(m2[:], d["p5"][:], AF.Square)
        rinv = spool.tile([128, 512], BF16, tag="rinv")
        nc.scalar.activation(rinv[:], g2[:], AF.Abs_reciprocal_sqrt, bias=1e-35)

        # V = q5 - mu^2: -I matmul folds m2 into the open q5 group, then
        # a single Act copy moves PSUM -> fp16 SBUF
        nc.tensor.matmul(d["q5"][:], In_m, m2[:], start=False, stop=True)
        gcv = opool.tile([128, 3, 512], F16, tag="gcv")
        nc.scalar.activation(gcv[:, 2, :], d["q5"][:], AF.Copy)
        d.update(g2=g2, rinv=rinv, gcv=gcv)

    def mid(i):
        # one slot after front: normalize + box3 matmuls + G
        d = st[i]
        s, g2, rinv, gcv = d["s"], d["g2"], d["rinv"], d["gcv"]
        rb = rinv[:].rearrange('p (o f) -> p o f', o=1).broadcast_to([128, 2, 512])
        uv = upool.tile([128, 2, 514], BF16, tag="uv")
        nc.vector.tensor_mul(uv[:, :, 1:513], d["gxy"][:], rb)
        nc.vector.tensor_copy(uv[:, :, 0:1], uv[:, :, 1:2])
        nc.vector.tensor_copy(uv[:, :, 513:514], uv[:, :, 512:513])

        # G = g2 * rinv -> fp16 (Pool, SBUF only)
        nc.gpsimd.tensor_mul(gcv[:, 0, :], g2[:], rinv[:])

        # ---- su|sv: box3 matmuls on the unit gradients (2D slices; the
        # matmul ISA rejects multi-free-dim operands) ----
        suv = psS.tile([128, 2, 512], F32, tag="suv")
        for o in range(2):
            nc.tensor.matmul(suv[:, o, :], MB(s, 1), uv[:, o, 1:513],
                             start=True, stop=False)
            nc.tensor.matmul(suv[:, o, :], MB(s, 0), uv[:, o, 0:512],
                             start=False, stop=False)
            nc.tensor.matmul(suv[:, o, :], MB(s, 0), uv[:, o, 2:514],
                             start=False, stop=True)
        d.update(uv=uv, suv=suv)

    T12_SPLIT = 384   # cols 0:384 on DVE, 384:512 on Pool

    def back(j):
        # separate tiles for the two column halves: a shared tile would
        # create a false WAW between the DVE and Pool writers
        d = st[j]
        uv, suv, gcv = d["uv"], d["suv"], d["gcv"]
        t12 = upool.tile([128, 2, 512], BF16, tag="t12")
        nc.vector.tensor_mul(t12[:], uv[:, :, 1:513], suv[:])
        # C = t1 + t2 -> fp16 (DVE bf16 2x)
        nc.vector.tensor_add(gcv[:, 1, :], t12[:, 0, :], t12[:, 1, :])

        ofs = TILE_OFS[j]
        m0, m1 = _valid_range(j)
        g0 = ofs + m0
        # stores on SP (loads ride the Act queue so prefetches never queue
        # behind stores); the LAST store goes on the Act queue — the final
        # SP-queue DMA is not reliably drained at program end
        eng = nc.scalar if j == N_TILES - 1 else nc.sync
        eng.dma_start(o_d[g0:g0 + (m1 - m0), :, :], gcv[m0:m1, :, :])
        st[j] = None

    load_x(0)
    nc.sync.dma_start(matsf_sb[:, 0:NF_MATS * 128],
                      matsf_d[:, 0:NF_MATS * 128].bitcast(F32R))
    load_x(1)
    prep(0)
    nc.sync.dma_start(matsb_sb[:], matsb_d[:])
    # emission order = scheduler priority: critical chain (sobel -> squares
    # -> rinv -> uv -> suv) first, then drains (t12/C/store), then prefetch
    for i in range(N_TILES + 2):
        if i < N_TILES:
            front_mm(i)
            front_elem(i)
        if 1 <= i <= N_TILES:
            mid(i - 1)
        if 2 <= i:
            back(i - 2)
        if i < N_TILES:
            if i + 1 < N_TILES:
                prep(i + 1)
            if i + 2 < N_TILES:
                load_x(i + 2)
            if i + 1 <= 3:
                c0, c1 = (i + 1) * NF_MATS * 128, (i + 2) * NF_MATS * 128
                nc.sync.dma_start(matsf_sb[:, c0:c1],
                                  matsf_d[:, c0:c1].bitcast(F32R))


_CACHE = {}


def _build():
    if "nc" in _CACHE:
        return _CACHE["nc"]
    nc = bacc.Bacc("TRN2", target_bir_lowering=False, debug=False)
    x_d = nc.dram_tensor("x", [ROWS, W], F32R, kind="ExternalInput").ap()
    o_d = nc.dram_tensor("O", [ROWS, 3, W], F16, kind="ExternalOutput").ap()
    import ml_dtypes
    arrf, arrb = _mats_const()
    matsf_d = nc.inline_tensor(arrf, name="matsf").ap()
    matsb_d = nc.inline_tensor(arrb.astype(ml_dtypes.bfloat16),
                               name="matsb").ap()
    # register a tiny-bias const AP for the rsqrt zero-guard
    _c = nc.alloc_sbuf_tensor("const-float32-1e-35", [128, 1], F32)
    nc.gpsimd.memset(_c.ap(), 1e-35)
    nc.const_aps.aps[(F32, 1e-35)] = _c.ap()
    with tile.TileContext(nc) as tc:
        with ExitStack() as ctx:
            _emit(ctx, tc, x_d, o_d, matsf_d, matsb_d)
    nc.compile()
    _CACHE["nc"] = nc
    return nc


def _run(inputs, trace=False):
    x = np.asarray(inputs["ivc_img"], np.float32)
    assert x.shape == (N_CORES * IPC, 1, H, W), x.shape
    nc = _build()
    in_maps = [
        {"x": np.ascontiguousarray(x[IPC * c:IPC * (c + 1), 0].reshape(ROWS, W))}
        for c in range(N_CORES)
    ]
    res = bass_utils.run_bass_kernel_spmd(
        nc, in_maps, core_ids=list(range(N_CORES)), trace=trace
    )
    outs = []
    for c in range(N_CORES):
        o = np.asarray(res.results[c]["O"], dtype=np.float32)
        o = o.reshape(IPC, H, 3, W).transpose(0, 2, 1, 3)
        outs.append(o)
    full = np.ascontiguousarray(np.concatenate(outs, axis=0))
    return full, res


def kernel(**inputs):
    full, _ = _run(inputs, trace=False)
    return full


def kernel_traced(**inputs):
    full, res = _run(inputs, trace=True)
    return full, res


# revision 3
# speedup vs baseline: 1.0200x; 1.0200x over previous
"""Trainium2 Bass kernel: NoiseEstimation (Sobel magnitude G, orientation
coherence C, 5x5 local variance V) over (16,1,512,512) fp32 input.

Math (trig-free):
  gx, gy = Sobel 3x3 cross-correlation, zero pad 1
  g2 = gx^2 + gy^2 (bf16);  r = abs_rsqrt(g2);  G = g2 * r
  u = gx*r, v = gy*r  (cos/sin of theta, bf16)
  C = u*box3c(u) + v*box3c(v)      box3c = (3x3 sum - center)/8, replicate
                                   pad; the -1/8 folds in via u^2+v^2=1
  V = box5(x^2)/25 - (box5(x)/25)^2     box5 = 5x5 sum, zero pad

Distribution: pure data parallel, 2 images per core on 8 NeuronCores.
Per core the 2 images are stacked into a [1024, 512] row space processed
as 9 overlapping 128-row tiles, software-pipelined across engines.

Engine assignment (balanced under the v2 timeline cost model; GPSIMD
cannot touch PSUM on hardware and its add/mult runs at 0.42 efficiency):
  Tensor : 18 matmuls/tile: sobel (5, fp32r, gy first), p5 = box5 of x
           via 5 shifted fp32r matmuls on a fp32 B5 band (replaces the
           hx scan - PE had headroom, DVE was the bottleneck), q5 from
           the hxx scan (bf16 B5), -I @ mu^2 folded into the open q5
           group (V = q5 - mu^2), su/sv box3 (6, bf16, shifted rhs)
  Act    : gxyb = bf16 copy of gxy PSUM (one pass; downstream squares /
           g2 / uv become bf16-2x DVE ops and the PSUM frees a slot
           early), rinv = Abs_rsqrt(g2), m2 = Square(p5), V = Copy(q5)
           -> fp16 (Copy/Square/Abs_rsqrt share one act table set)
  DVE    : hxx = 5-window running sum scan of x^2, pq2 = gxyb*gxyb,
           g2 = p2+q2, uv = gxyb*rinv (all bf16 2x), uv edge copies,
           t12 = uv*suv (PSUM mixed - only DVE can), C = t1+t2 -> fp16
  Pool   : xx = x^2 (SBUF fp32 in), G = g2*rinv -> fp16, x pad memsets
  SP/Act : stores ride SP, input prefetches ride the Act DGE queue (no
           waits -> no head-of-line blocking); the LAST store issues on
           the Act queue - the final SP-queue DMA is not reliably
           drained at program end (silent tail corruption otherwise)

Output is staged in fp16 (halves store traffic vs fp32; host upcasts,
well inside the 2e-2 tolerance). PSUM: gxy[2]x2 double-buffered, p5[1],
q5[1], suv[2] = 8 banks. PE warmup burst rides out the cold p-states.
TimelineSim (v2 cost model): ~50.1us/core vs 87.2us for the original
Pool-heavy kernel (1.74x); CoreSim (v1): ~47.4us. Verified on the 8
axon trn2 cores: rel err 5.75e-3 (gate 2e-2). Steady state is paced by
the PE at ~3.8us/tile (18 matmuls); pipeline fill (~6us) + drain (~8us)
are the remaining overhead.
"""

import numpy as np
from contextlib import ExitStack

import concourse.bass as bass
import concourse.bacc as bacc
import concourse.tile as tile
import concourse.mybir as mybir
from concourse import bass_utils

F32 = mybir.dt.float32
F32R = mybir.dt.float32r
F16 = mybir.dt.float16
BF16 = mybir.dt.bfloat16
AL = mybir.AluOpType
AF = mybir.ActivationFunctionType

H = 512
W = 512
N_CORES = 8
IPC = 2                    # images per core
ROWS = IPC * H             # 1024 stacked rows per core
TILE_OFS = [0, 124, 248, 372, 496, 620, 744, 868, 896]
N_TILES = len(TILE_OFS)
# tile -> band-matrix set (0=t0 top, 1=interior, 2=image boundary, 3=bottom)
TILE_SET = [0, 1, 1, 1, 2, 1, 1, 1, 3]
SET_OFS = [0, 124, 496, 896]
N_SETS = 4
NF_MATS = 5   # fp32 per set: V121p, V121n, Vd1, Vd2, B5f
NB_MATS = 3   # bf16 per set: B3, B3c, B5


def _valid_range(ti):
    """Valid output partition range [m0, m1) for tile ti."""
    if ti == 0:
        return 0, 126
    if ti == N_TILES - 1:
        start_g = TILE_OFS[ti - 1] + 126   # first row not covered by prev tile
        return start_g - TILE_OFS[ti], 128
    return 2, 126


def _build_mat_set(ofs):
    """Banded 128x128 lhsT matrices for a tile at row offset `ofs`.

    lhsT[k, m] = weight of tile input row k contributing to output row m.
    Image edges (zero pad for sobel/box5, replicate for box3) are encoded
    per-row; tiles spanning the two-image boundary get block-diagonal bands.
    """
    fmats = [np.zeros((128, 128), np.float32) for _ in range(NF_MATS)]
    bmats = [np.zeros((128, 128), np.float32) for _ in range(NB_MATS)]
    V121p, V121n, Vd1, Vd2, B5f = fmats
    B3, B3c, B5 = bmats
    for m in range(128):
        g = ofs + m
        img = g // H
        if img >= IPC:
            continue
        lo, hi = img * H, img * H + H - 1

        def add(mat, d, w, rep=False):
            gt = g + d
            if gt < lo or gt > hi:
                if not rep:
                    return
                gt = min(max(gt, lo), hi)
            k = gt - ofs
            if 0 <= k < 128:
                mat[k, m] += w

        for d, wgt in ((-1, 1.0), (0, 2.0), (1, 1.0)):
            add(V121p, d, wgt)
            add(V121n, d, -wgt)
        for d, wgt in ((-1, -1.0), (1, 1.0)):
            add(Vd1, d, wgt)
            add(Vd2, d, 2.0 * wgt)
        for d in (-2, -1, 0, 1, 2):
            add(B5, d, 0.04)
            add(B5f, d, 0.04)
        for d in (-1, 0, 1):
            add(B3, d, 0.125, rep=True)
            add(B3c, d, 0.125, rep=True)
        B3c[m, m] -= 0.125  # folds C's "- 1/8" via u^2 + v^2 = 1
    return fmats, bmats


def _mats_const():
    """fp32 sobel mats [128, 4*4*128]; bf16 mats [128, (4*3+1)*128]
    (B3, B3c, B5 per set + trailing -I)."""
    arrf = np.zeros((128, N_SETS * NF_MATS * 128), np.float32)
    arrb = np.zeros((128, (N_SETS * NB_MATS + 1) * 128), np.float32)
    for s, ofs in enumerate(SET_OFS):
        fmats, bmats = _build_mat_set(ofs)
        for f, mat in enumerate(fmats):
            b = s * NF_MATS + f
            arrf[:, b * 128:(b + 1) * 128] = mat
        for f, mat in enumerate(bmats):
            b = s * NB_MATS + f
            arrb[:, b * 128:(b + 1) * 128] = mat
    arrb[:, N_SETS * NB_MATS * 128:] = -np.eye(128, dtype=np.float32)
    return arrf, arrb


def _emit(ctx: ExitStack, tc: "tile.TileContext", x_d, o_d, matsf_d, matsb_d):
    nc = tc.nc
    mpool = ctx.enter_context(tc.tile_pool(name="mats", bufs=1))
    xpool = ctx.enter_context(tc.tile_pool(name="xp", bufs=5))
    spool = ctx.enter_context(tc.tile_pool(name="sp", bufs=3))
    upool = ctx.enter_context(tc.tile_pool(name="up", bufs=3))
    opool = ctx.enter_context(tc.tile_pool(name="op", bufs=4))
    psA = ctx.enter_context(tc.tile_pool(name="psA", bufs=2, space="PSUM"))
    psP = ctx.enter_context(tc.tile_pool(name="psP", bufs=1, space="PSUM"))
    psQ = ctx.enter_context(tc.tile_pool(name="psQ", bufs=1, space="PSUM"))
    psS = ctx.enter_context(tc.tile_pool(name="psS", bufs=1, space="PSUM"))

    matsf_sb = mpool.tile([128, N_SETS * NF_MATS * 128], F32R, tag="matsf")
    matsb_sb = mpool.tile([128, (N_SETS * NB_MATS + 1) * 128], BF16, tag="matsb")

    def MF(s, f):
        b = s * NF_MATS + f
        return matsf_sb[:, b * 128:(b + 1) * 128]

    def MB(s, f):
        b = s * NB_MATS + f
        return matsb_sb[:, b * 128:(b + 1) * 128]

    In_m = matsb_sb[:, N_SETS * NB_MATS * 128:]

    # force the single act table set (abs_reciprocal_sqrt_and_small:
    # abs_rsqrt / square / copy)
    scratch = mpool.tile([128, 8], F32, tag="scr")
    nc.gpsimd.memset(scratch[:], 1.0)
    nc.scalar.activation(scratch[:, 0:4], scratch[:, 4:8],
                         AF.Abs_reciprocal_sqrt, bias=1e-35)

    # PE warmup: the PE clock ramps only while busy (HAM gate); a burst of
    # dummy N=128 matmuls on zeroed weights from t~0 rides out the cold
    # p-states during the initial DMA window so tile 0 runs at full rate.
    wmat = mpool.tile([128, 128], BF16, tag="wm")
    nc.gpsimd.memset(wmat[:], 0.0)
    wps = psP.tile([128, 512], F32, tag="p5")
    NWARM = 20
    for k in range(NWARM):
        nc.tensor.matmul(wps[:, 0:128], wmat[:], wmat[:],
                         start=(k == 0), stop=(k == NWARM - 1))

    # per-tile state carried across the software pipeline
    st = [None] * N_TILES
    xq = [None] * N_TILES
    sq = [None] * N_TILES

    def load_x(i):
        # x tile: [4 zero | 512 | 4 zero] cols; issued two slots ahead.
        # The pad columns are memset once per physical buffer (first 4
        # tiles): the DMA only ever writes cols 4:516, so they stay zero.
        x_t = xpool.tile([128, 520], F32R, tag="x")
        nc.scalar.dma_start(x_t[:, 4:516], x_d[TILE_OFS[i]:TILE_OFS[i] + 128, :])
        nc.gpsimd.memset(x_t[:, 0:4].bitcast(F32), 0.0)
        nc.gpsimd.memset(x_t[:, 516:520].bitcast(F32), 0.0)
        xq[i] = x_t

    def prep(i):
        # xx = x^2 (Act, bf16; zero pads square to zero), then the two
        # horizontal 5-window running sums as DVE scans:
        # state = (d0[t] + state) - d1[t]  ->  out[t] = sum x[t+1..t+5]
        x_t = xq[i]
        xf = x_t.bitcast(F32)
        xx_t = spool.tile([128, 518], BF16, tag="xx")
        nc.gpsimd.tensor_mul(xx_t[:], xf[:, 0:518], xf[:, 0:518])
        hxx_t = spool.tile([128, 513], BF16, tag="hxx")
        nc.vector.tensor_tensor_scan(hxx_t[:], xx_t[:, 5:518], xx_t[:, 0:513],
                                     xx_t[:, 4:5], AL.add, AL.subtract)
        sq[i] = hxx_t

    def front_mm(i):
        # PE-only: no upstream deps beyond x/hx/hxx -> issue first per slot.
        # gy first: its Square can overlap the gx matmuls, shortening the
        # rinv critical chain.
        s = TILE_SET[i]
        x_t = xq[i]

        def xr(j):
            return x_t[:, 4 + j:4 + j + 512]

        # ---- sobel via banded matmuls -> gxy PSUM [128, 2, 512] ----
        gxy = psA.tile([128, 2, 512], F32, tag="gxy")
        nc.tensor.matmul(gxy[:, 1, :], MF(s, 2), xr(-1), start=True, stop=False)
        nc.tensor.matmul(gxy[:, 1, :], MF(s, 2), xr(+1), start=False, stop=False)
        nc.tensor.matmul(gxy[:, 1, :], MF(s, 3), xr(0), start=False, stop=True)
        nc.tensor.matmul(gxy[:, 0, :], MF(s, 0), xr(+1), start=True, stop=False)
        nc.tensor.matmul(gxy[:, 0, :], MF(s, 1), xr(-1), start=False, stop=True)

        hxx_t = sq[i]

        # ---- box5 sums: p5 directly from x via 5 shifted fp32r matmuls
        # (kills the hx scan on the DVE); q5 from the hxx scan ----
        p5 = psP.tile([128, 512], F32, tag="p5")
        for jj, j in enumerate((-2, -1, 0, 1, 2)):
            nc.tensor.matmul(p5[:], MF(s, 4), xr(j), start=(jj == 0),
                             stop=(jj == 4))
        q5 = psQ.tile([128, 512], F32, tag="q5")
        nc.tensor.matmul(q5[:], MB(s, 2), hxx_t[:, 1:513], start=True, stop=False)

        st[i] = dict(s=s, gxy=gxy, p5=p5, q5=q5)

    def front_elem(i):
        # G-chain head + V; rinv lands at slot end, its consumers (uv, G)
        # run next slot so the spill is absorbed.
        d = st[i]
        # one Act pass converts gxy PSUM -> bf16 SBUF; squares, g2 and uv
        # then run as cheap bf16-2x DVE ops, and the PSUM frees a slot
        # earlier (no write-after-read loop through uv)
        gxyb = spool.tile([128, 2, 512], BF16, tag="gxyb")
        nc.scalar.activation(gxyb[:], d["gxy"][:], AF.Copy)
        pq2 = spool.tile([128, 2, 512], BF16, tag="pq2")
        nc.vector.tensor_mul(pq2[:], gxyb[:], gxyb[:])
        g2 = spool.tile([128, 512], BF16, tag="g2")
        nc.vector.tensor_add(g2[:], pq2[:, 0, :], pq2[:, 1, :])
        m2 = spool.tile([128, 512], BF16, tag="m2")
        nc.scalar.activation(m2[:], d["p5"][:], AF.Square)
        rinv = spool.tile([128, 512], BF16, tag="rinv")
        nc.scalar.activation(rinv[:], g2[:], AF.Abs_reciprocal_sqrt, bias=1e-35)

        # V = q5 - mu^2: -I matmul folds m2 into the open q5 group, then
        # a single Act copy moves PSUM -> fp16 SBUF
        nc.tensor.matmul(d["q5"][:], In_m, m2[:], start=False, stop=True)
        gcv = opool.tile([128, 3, 512], F16, tag="gcv")
        nc.scalar.activation(gcv[:, 2, :], d["q5"][:], AF.Copy)
        d.update(g2=g2, rinv=rinv, gcv=gcv, gxyb=gxyb)

    def mid(i):
        # one slot after front: normalize + box3 matmuls + G
        d = st[i]
        s, g2, rinv, gcv = d["s"], d["g2"], d["rinv"], d["gcv"]
        rb = rinv[:].rearrange('p (o f) -> p o f', o=1).broadcast_to([128, 2, 512])
        uv = upool.tile([128, 2, 514], BF16, tag="uv")
        nc.vector.tensor_mul(uv[:, :, 1:513], d["gxyb"][:], rb)
        nc.vector.tensor_copy(uv[:, :, 0:1], uv[:, :, 1:2])
        nc.vector.tensor_copy(uv[:, :, 513:514], uv[:, :, 512:513])

        # G = g2 * rinv -> fp16 (Pool, SBUF only)
        nc.gpsimd.tensor_mul(gcv[:, 0, :], g2[:], rinv[:])

        # ---- su|sv: box3 matmuls on the unit gradients (2D slices; the
        # matmul ISA rejects multi-free-dim operands) ----
        suv = psS.tile([128, 2, 512], F32, tag="suv")
        for o in range(2):
            nc.tensor.matmul(suv[:, o, :], MB(s, 1), uv[:, o, 1:513],
                             start=True, stop=False)
            nc.tensor.matmul(suv[:, o, :], MB(s, 0), uv[:, o, 0:512],
                             start=False, stop=False)
            nc.tensor.matmul(suv[:, o, :], MB(s, 0), uv[:, o, 2:514],
                             start=False, stop=True)
        d.update(uv=uv, suv=suv)

    T12_SPLIT = 384   # cols 0:384 on DVE, 384:512 on Pool

    def back(j):
        # separate tiles for the two column halves: a shared tile would
        # create a false WAW between the DVE and Pool writers
        d = st[j]
        uv, suv, gcv = d["uv"], d["suv"], d["gcv"]
        t12 = upool.tile([128, 2, 512], BF16, tag="t12")
        nc.vector.tensor_mul(t12[:], uv[:, :, 1:513], suv[:])
        # C = t1 + t2 -> fp16 (DVE bf16 2x)
        nc.vector.tensor_add(gcv[:, 1, :], t12[:, 0, :], t12[:, 1, :])

        ofs = TILE_OFS[j]
        m0, m1 = _valid_range(j)
        g0 = ofs + m0
        # stores on SP (loads ride the Act queue so prefetches never queue
        # behind stores); the LAST store goes on the Act queue — the final
        # SP-queue DMA is not reliably drained at program end
        eng = nc.scalar if j == N_TILES - 1 else nc.sync
        eng.dma_start(o_d[g0:g0 + (m1 - m0), :, :], gcv[m0:m1, :, :])
        st[j] = None

    load_x(0)
    nc.sync.dma_start(matsf_sb[:, 0:NF_MATS * 128],
                      matsf_d[:, 0:NF_MATS * 128].bitcast(F32R))
    load_x(1)
    prep(0)
    nc.sync.dma_start(matsb_sb[:], matsb_d[:])
    # emission order = scheduler priority: critical chain (sobel -> squares
    # -> rinv -> uv -> suv) first, then drains (t12/C/store), then prefetch
    for i in range(N_TILES + 2):
        if i < N_TILES:
            front_mm(i)
            front_elem(i)
        if 1 <= i <= N_TILES:
            mid(i - 1)
        if 2 <= i:
            back(i - 2)
        if i < N_TILES:
            if i + 1 < N_TILES:
                prep(i + 1)
            if i + 2 < N_TILES:
                load_x(i + 2)
            if i + 1 <= 3:
                c0, c1 = (i + 1) * NF_MATS * 128, (i + 2) * NF_MATS * 128
                nc.sync.dma_start(matsf_sb[:, c0:c1],
                                  matsf_d[:, c0:c1].bitcast(F32R))


_CACHE = {}


def _build():
    if "nc" in _CACHE:
        return _CACHE["nc"]
    nc = bacc.Bacc("TRN2", target_bir_lowering=False, debug=False)
    x_d = nc.dram_tensor("x", [ROWS, W], F32R, kind="ExternalInput").ap()
    o_d = nc.dram_tensor("O", [ROWS, 3, W], F16, kind="ExternalOutput").ap()
    import ml_dtypes
    arrf, arrb = _mats_const()
    matsf_d = nc.inline_tensor(arrf, name="matsf").ap()
    matsb_d = nc.inline_tensor(arrb.astype(ml_dtypes.bfloat16),
                               name="matsb").ap()
    # register a tiny-bias const AP for the rsqrt zero-guard
    _c = nc.alloc_sbuf_tensor("const-float32-1e-35", [128, 1], F32)
    nc.gpsimd.memset(_c.ap(), 1e-35)
    nc.const_aps.aps[(F32, 1e-35)] = _c.ap()
    with tile.TileContext(nc) as tc:
        with ExitStack() as ctx:
            _emit(ctx, tc, x_d, o_d, matsf_d, matsb_d)
    nc.compile()
    _CACHE["nc"] = nc
    return nc


def _run(inputs, trace=False):
    x = np.asarray(inputs["ivc_img"], np.float32)
    assert x.shape == (N_CORES * IPC, 1, H, W), x.shape
    nc = _build()
    in_maps = [
        {"x": np.ascontiguousarray(x[IPC * c:IPC * (c + 1), 0].reshape(ROWS, W))}
        for c in range(N_CORES)
    ]
    res = bass_utils.run_bass_kernel_spmd(
        nc, in_maps, core_ids=list(range(N_CORES)), trace=trace
    )
    outs = []
    for c in range(N_CORES):
        o = np.asarray(res.results[c]["O"], dtype=np.float32)
        o = o.reshape(IPC, H, 3, W).transpose(0, 2, 1, 3)
        outs.append(o)
    full = np.ascontiguousarray(np.concatenate(outs, axis=0))
    return full, res


def kernel(**inputs):
    full, _ = _run(inputs, trace=False)
    return full


def kernel_traced(**inputs):
    full, res = _run(inputs, trace=True)
    return full, res


# revision 4
# speedup vs baseline: 1.0236x; 1.0036x over previous
"""Trainium2 Bass kernel: NoiseEstimation (Sobel magnitude G, orientation
coherence C, 5x5 local variance V) over (16,1,512,512) fp32 input.

Math (trig-free):
  gx, gy = Sobel 3x3 cross-correlation, zero pad 1
  g2 = gx^2 + gy^2 (bf16);  r = abs_rsqrt(g2);  G = g2 * r
  u = gx*r, v = gy*r  (cos/sin of theta, bf16)
  C = u*box3c(u) + v*box3c(v)      box3c = (3x3 sum - center)/8, replicate
                                   pad; the -1/8 folds in via u^2+v^2=1
  V = box5(x^2)/25 - (box5(x)/25)^2     box5 = 5x5 sum, zero pad

Distribution: pure data parallel, 2 images per core on 8 NeuronCores.
Per core the 2 images are stacked into a [1024, 512] row space processed
as 9 overlapping 128-row tiles, software-pipelined across engines.

Engine assignment (balanced under the v2 timeline cost model; GPSIMD
cannot touch PSUM on hardware and its add/mult runs at 0.42 efficiency):
  Tensor : 18 matmuls/tile: sobel (5, fp32r, gy first), p5 = box5 of x
           via 5 shifted fp32r matmuls on a fp32 B5 band (replaces the
           hx scan - PE had headroom, DVE was the bottleneck), q5 from
           the hxx scan (bf16 B5), -I @ mu^2 folded into the open q5
           group (V = q5 - mu^2), su/sv box3 (6, bf16, shifted rhs)
  Act    : gxyb = bf16 copy of gxy PSUM (one pass; downstream squares /
           g2 / uv become bf16-2x DVE ops and the PSUM frees a slot
           early), rinv = Abs_rsqrt(g2), m2 = Square(p5), V = Copy(q5)
           -> fp16 (Copy/Square/Abs_rsqrt share one act table set)
  DVE    : hxx = 5-window running sum scan of x^2, pq2 = gxyb*gxyb,
           g2 = p2+q2, uv = gxyb*rinv (all bf16 2x), uv edge copies,
           t12 = uv*suv (PSUM mixed - only DVE can), C = t1+t2 -> fp16
  Pool   : xx = x^2 (SBUF fp32 in), G = g2*rinv -> fp16, x pad memsets
  SP/Act : stores ride SP, input prefetches ride the Act DGE queue (no
           waits -> no head-of-line blocking); the LAST store issues on
           the Act queue - the final SP-queue DMA is not reliably
           drained at program end (silent tail corruption otherwise)

Output is staged in fp16 (halves store traffic vs fp32; host upcasts,
well inside the 2e-2 tolerance). PSUM: gxy[2]x2 double-buffered, p5[1],
q5[1], suv[2] = 8 banks. PE warmup burst rides out the cold p-states.
Drain-specific tweaks: the last tile's suv borrows a free psA-ring
buffer (dodges the single-psS wait behind t12(N-2)) and its uv/t12 run
per-channel so the two channel sub-chains pipeline through the drain.

TimelineSim (v2 cost model): ~49.1us/core vs 87.2us for the original
Pool-heavy kernel (1.78x); CoreSim (v1): ~46.4us. Verified on the 8
axon trn2 cores: rel err 5.75e-3 (gate 2e-2). Steady state is paced by
the PE at ~3.8us/tile (18 matmuls); pipeline fill (~5us) + drain (~7us)
are the remaining overhead.
"""

import numpy as np
from contextlib import ExitStack

import concourse.bass as bass
import concourse.bacc as bacc
import concourse.tile as tile
import concourse.mybir as mybir
from concourse import bass_utils

F32 = mybir.dt.float32
F32R = mybir.dt.float32r
F16 = mybir.dt.float16
BF16 = mybir.dt.bfloat16
AL = mybir.AluOpType
AF = mybir.ActivationFunctionType

H = 512
W = 512
N_CORES = 8
IPC = 2                    # images per core
ROWS = IPC * H             # 1024 stacked rows per core
TILE_OFS = [0, 124, 248, 372, 496, 620, 744, 868, 896]
N_TILES = len(TILE_OFS)
# tile -> band-matrix set (0=t0 top, 1=interior, 2=image boundary, 3=bottom)
TILE_SET = [0, 1, 1, 1, 2, 1, 1, 1, 3]
SET_OFS = [0, 124, 496, 896]
N_SETS = 4
NF_MATS = 5   # fp32 per set: V121p, V121n, Vd1, Vd2, B5f
NB_MATS = 3   # bf16 per set: B3, B3c, B5


def _valid_range(ti):
    """Valid output partition range [m0, m1) for tile ti."""
    if ti == 0:
        return 0, 126
    if ti == N_TILES - 1:
        start_g = TILE_OFS[ti - 1] + 126   # first row not covered by prev tile
        return start_g - TILE_OFS[ti], 128
    return 2, 126


def _build_mat_set(ofs):
    """Banded 128x128 lhsT matrices for a tile at row offset `ofs`.

    lhsT[k, m] = weight of tile input row k contributing to output row m.
    Image edges (zero pad for sobel/box5, replicate for box3) are encoded
    per-row; tiles spanning the two-image boundary get block-diagonal bands.
    """
    fmats = [np.zeros((128, 128), np.float32) for _ in range(NF_MATS)]
    bmats = [np.zeros((128, 128), np.float32) for _ in range(NB_MATS)]
    V121p, V121n, Vd1, Vd2, B5f = fmats
    B3, B3c, B5 = bmats
    for m in range(128):
        g = ofs + m
        img = g // H
        if img >= IPC:
            continue
        lo, hi = img * H, img * H + H - 1

        def add(mat, d, w, rep=False):
            gt = g + d
            if gt < lo or gt > hi:
                if not rep:
                    return
                gt = min(max(gt, lo), hi)
            k = gt - ofs
            if 0 <= k < 128:
                mat[k, m] += w

        for d, wgt in ((-1, 1.0), (0, 2.0), (1, 1.0)):
            add(V121p, d, wgt)
            add(V121n, d, -wgt)
        for d, wgt in ((-1, -1.0), (1, 1.0)):
            add(Vd1, d, wgt)
            add(Vd2, d, 2.0 * wgt)
        for d in (-2, -1, 0, 1, 2):
            add(B5, d, 0.04)
            add(B5f, d, 0.04)
        for d in (-1, 0, 1):
            add(B3, d, 0.125, rep=True)
            add(B3c, d, 0.125, rep=True)
        B3c[m, m] -= 0.125  # folds C's "- 1/8" via u^2 + v^2 = 1
    return fmats, bmats


def _mats_const():
    """fp32 sobel mats [128, 4*4*128]; bf16 mats [128, (4*3+1)*128]
    (B3, B3c, B5 per set + trailing -I)."""
    arrf = np.zeros((128, N_SETS * NF_MATS * 128), np.float32)
    arrb = np.zeros((128, (N_SETS * NB_MATS + 1) * 128), np.float32)
    for s, ofs in enumerate(SET_OFS):
        fmats, bmats = _build_mat_set(ofs)
        for f, mat in enumerate(fmats):
            b = s * NF_MATS + f
            arrf[:, b * 128:(b + 1) * 128] = mat
        for f, mat in enumerate(bmats):
            b = s * NB_MATS + f
            arrb[:, b * 128:(b + 1) * 128] = mat
    arrb[:, N_SETS * NB_MATS * 128:] = -np.eye(128, dtype=np.float32)
    return arrf, arrb


def _emit(ctx: ExitStack, tc: "tile.TileContext", x_d, o_d, matsf_d, matsb_d):
    nc = tc.nc
    mpool = ctx.enter_context(tc.tile_pool(name="mats", bufs=1))
    xpool = ctx.enter_context(tc.tile_pool(name="xp", bufs=5))
    spool = ctx.enter_context(tc.tile_pool(name="sp", bufs=3))
    upool = ctx.enter_context(tc.tile_pool(name="up", bufs=3))
    opool = ctx.enter_context(tc.tile_pool(name="op", bufs=4))
    psA = ctx.enter_context(tc.tile_pool(name="psA", bufs=2, space="PSUM"))
    psP = ctx.enter_context(tc.tile_pool(name="psP", bufs=1, space="PSUM"))
    psQ = ctx.enter_context(tc.tile_pool(name="psQ", bufs=1, space="PSUM"))
    psS = ctx.enter_context(tc.tile_pool(name="psS", bufs=1, space="PSUM"))

    matsf_sb = mpool.tile([128, N_SETS * NF_MATS * 128], F32R, tag="matsf")
    matsb_sb = mpool.tile([128, (N_SETS * NB_MATS + 1) * 128], BF16, tag="matsb")

    def MF(s, f):
        b = s * NF_MATS + f
        return matsf_sb[:, b * 128:(b + 1) * 128]

    def MB(s, f):
        b = s * NB_MATS + f
        return matsb_sb[:, b * 128:(b + 1) * 128]

    In_m = matsb_sb[:, N_SETS * NB_MATS * 128:]

    # per-tile state carried across the software pipeline
    st = [None] * N_TILES
    xq = [None] * N_TILES
    sq = [None] * N_TILES

    def load_x(i):
        # x tile: [4 zero | 512 | 4 zero] cols; issued two slots ahead.
        # The pad columns are memset once per physical buffer (first 4
        # tiles): the DMA only ever writes cols 4:516, so they stay zero.
        x_t = xpool.tile([128, 520], F32R, tag="x")
        nc.scalar.dma_start(x_t[:, 4:516], x_d[TILE_OFS[i]:TILE_OFS[i] + 128, :])
        nc.gpsimd.memset(x_t[:, 0:4].bitcast(F32), 0.0)
        nc.gpsimd.memset(x_t[:, 516:520].bitcast(F32), 0.0)
        xq[i] = x_t

    def prep(i):
        # xx = x^2 (Act, bf16; zero pads square to zero), then the two
        # horizontal 5-window running sums as DVE scans:
        # state = (d0[t] + state) - d1[t]  ->  out[t] = sum x[t+1..t+5]
        x_t = xq[i]
        xf = x_t.bitcast(F32)
        xx_t = spool.tile([128, 518], BF16, tag="xx")
        nc.gpsimd.tensor_mul(xx_t[:], xf[:, 0:518], xf[:, 0:518])
        hxx_t = spool.tile([128, 513], BF16, tag="hxx")
        nc.vector.tensor_tensor_scan(hxx_t[:], xx_t[:, 5:518], xx_t[:, 0:513],
                                     xx_t[:, 4:5], AL.add, AL.subtract)
        sq[i] = hxx_t

    def front_mm(i):
        # PE-only: no upstream deps beyond x/hx/hxx -> issue first per slot.
        # gy first: its Square can overlap the gx matmuls, shortening the
        # rinv critical chain.
        s = TILE_SET[i]
        x_t = xq[i]

        def xr(j):
            return x_t[:, 4 + j:4 + j + 512]

        # ---- sobel via banded matmuls -> gxy PSUM [128, 2, 512] ----
        gxy = psA.tile([128, 2, 512], F32, tag="gxy")
        nc.tensor.matmul(gxy[:, 1, :], MF(s, 2), xr(-1), start=True, stop=False)
        nc.tensor.matmul(gxy[:, 1, :], MF(s, 2), xr(+1), start=False, stop=False)
        nc.tensor.matmul(gxy[:, 1, :], MF(s, 3), xr(0), start=False, stop=True)
        nc.tensor.matmul(gxy[:, 0, :], MF(s, 0), xr(+1), start=True, stop=False)
        nc.tensor.matmul(gxy[:, 0, :], MF(s, 1), xr(-1), start=False, stop=True)

        hxx_t = sq[i]

        # ---- box5 sums: p5 directly from x via 5 shifted fp32r matmuls
        # (kills the hx scan on the DVE); q5 from the hxx scan ----
        p5 = psP.tile([128, 512], F32, tag="p5")
        for jj, j in enumerate((-2, -1, 0, 1, 2)):
            nc.tensor.matmul(p5[:], MF(s, 4), xr(j), start=(jj == 0),
                             stop=(jj == 4))
        q5 = psQ.tile([128, 512], F32, tag="q5")
        nc.tensor.matmul(q5[:], MB(s, 2), hxx_t[:, 1:513], start=True, stop=False)

        st[i] = dict(s=s, gxy=gxy, p5=p5, q5=q5)

    def front_elem(i):
        # G-chain head + V; rinv lands at slot end, its consumers (uv, G)
        # run next slot so the spill is absorbed.
        d = st[i]
        # one Act pass converts gxy PSUM -> bf16 SBUF; squares, g2 and uv
        # then run as cheap bf16-2x DVE ops, and the PSUM frees a slot
        # earlier (no write-after-read loop through uv)
        gxyb = spool.tile([128, 2, 512], BF16, tag="gxyb")
        nc.scalar.activation(gxyb[:], d["gxy"][:], AF.Copy)
        pq2 = spool.tile([128, 2, 512], BF16, tag="pq2")
        nc.vector.tensor_mul(pq2[:], gxyb[:], gxyb[:])
        g2 = spool.tile([128, 512], BF16, tag="g2")
        nc.vector.tensor_add(g2[:], pq2[:, 0, :], pq2[:, 1, :])
        m2 = spool.tile([128, 512], BF16, tag="m2")
        nc.scalar.activation(m2[:], d["p5"][:], AF.Square)
        rinv = spool.tile([128, 512], BF16, tag="rinv")
        nc.scalar.activation(rinv[:], g2[:], AF.Abs_reciprocal_sqrt, bias=1e-35)

        # V = q5 - mu^2: -I matmul folds m2 into the open q5 group, then
        # a single Act copy moves PSUM -> fp16 SBUF
        nc.tensor.matmul(d["q5"][:], In_m, m2[:], start=False, stop=True)
        gcv = opool.tile([128, 3, 512], F16, tag="gcv")
        nc.scalar.activation(gcv[:, 2, :], d["q5"][:], AF.Copy)
        d.update(g2=g2, rinv=rinv, gcv=gcv, gxyb=gxyb)

    def mid(i):
        # one slot after front: normalize + box3 matmuls + G
        d = st[i]
        s, g2, rinv, gcv = d["s"], d["g2"], d["rinv"], d["gcv"]
        if i == N_TILES - 1:
            # last tile: per-channel in SEPARATE tiles so the u-chain
            # (suv-o0, t12-u) is not dep-coupled to the v writes during
            # the pipeline drain
            uv0 = upool.tile([128, 514], BF16, tag="uv80")
            uv1 = upool.tile([128, 514], BF16, tag="uv81")
            uv = [uv0, uv1]
            for o in range(2):
                nc.vector.tensor_mul(uv[o][:, 1:513], d["gxyb"][:, o, :],
                                     rinv[:])
                nc.vector.tensor_copy(uv[o][:, 0:1], uv[o][:, 1:2])
                nc.vector.tensor_copy(uv[o][:, 513:514], uv[o][:, 512:513])
        else:
            uv = upool.tile([128, 2, 514], BF16, tag="uv")
            rb = rinv[:].rearrange('p (o f) -> p o f', o=1).broadcast_to(
                [128, 2, 512])
            nc.vector.tensor_mul(uv[:, :, 1:513], d["gxyb"][:], rb)
            nc.vector.tensor_copy(uv[:, :, 0:1], uv[:, :, 1:2])
            nc.vector.tensor_copy(uv[:, :, 513:514], uv[:, :, 512:513])

        # G = g2 * rinv -> fp16 (Pool, SBUF only)
        nc.gpsimd.tensor_mul(gcv[:, 0, :], g2[:], rinv[:])

        # ---- su|sv: box3 matmuls on the unit gradients (2D slices; the
        # matmul ISA rejects multi-free-dim operands) ----
        # the LAST tile's suv borrows a psA-ring buffer (its gxy PSUM is
        # already consumed) so the drain never waits on the single psS
        # buffer behind t12(N-2)'s read
        if i == N_TILES - 1:
            suv = psA.tile([128, 2, 512], F32, tag="gxy")
        else:
            suv = psS.tile([128, 2, 512], F32, tag="suv")
        for o in range(2):
            uo = uv[o] if i == N_TILES - 1 else uv[:, o, :]
            nc.tensor.matmul(suv[:, o, :], MB(s, 1), uo[:, 1:513],
                             start=True, stop=False)
            nc.tensor.matmul(suv[:, o, :], MB(s, 0), uo[:, 0:512],
                             start=False, stop=False)
            nc.tensor.matmul(suv[:, o, :], MB(s, 0), uo[:, 2:514],
                             start=False, stop=True)
        d.update(uv=uv, suv=suv)

    T12_SPLIT = 384   # cols 0:384 on DVE, 384:512 on Pool

    def back(j):
        # separate tiles for the two column halves: a shared tile would
        # create a false WAW between the DVE and Pool writers
        d = st[j]
        uv, suv, gcv = d["uv"], d["suv"], d["gcv"]
        t12 = upool.tile([128, 2, 512], BF16, tag="t12")
        if j == N_TILES - 1:
            nc.vector.tensor_mul(t12[:, 0, :], uv[0][:, 1:513], suv[:, 0, :])
            nc.vector.tensor_mul(t12[:, 1, :], uv[1][:, 1:513], suv[:, 1, :])
        else:
            nc.vector.tensor_mul(t12[:], uv[:, :, 1:513], suv[:])
        # C = t1 + t2 -> fp16 (DVE bf16 2x)
        nc.vector.tensor_add(gcv[:, 1, :], t12[:, 0, :], t12[:, 1, :])

        ofs = TILE_OFS[j]
        m0, m1 = _valid_range(j)
        g0 = ofs + m0
        # stores on SP (loads ride the Act queue so prefetches never queue
        # behind stores); the LAST store goes on the Act queue — the final
        # SP-queue DMA is not reliably drained at program end
        eng = nc.scalar if j == N_TILES - 1 else nc.sync
        eng.dma_start(o_d[g0:g0 + (m1 - m0), :, :], gcv[m0:m1, :, :])
        st[j] = None

    load_x(0)
    nc.sync.dma_start(matsf_sb[:, 0:NF_MATS * 128],
                      matsf_d[:, 0:NF_MATS * 128].bitcast(F32R))
    load_x(1)
    nc.sync.dma_start(matsb_sb[:], matsb_d[:])

    # force the single act table set (abs_rsqrt / square / copy) - emitted
    # AFTER the loads so the x0 DMA dispatch is not stuck behind it on the
    # Act SEQ
    scratch = mpool.tile([128, 8], F32, tag="scr")
    nc.gpsimd.memset(scratch[:], 1.0)
    nc.scalar.activation(scratch[:, 0:4], scratch[:, 4:8],
                         AF.Abs_reciprocal_sqrt, bias=1e-35)

    # PE warmup: the PE clock ramps only while busy (HAM gate); a burst of
    # dummy N=128 matmuls on zeroed weights rides out the cold p-states
    # during the initial DMA window so tile 0 runs at full rate.
    wmat = mpool.tile([128, 128], BF16, tag="wm")
    nc.gpsimd.memset(wmat[:], 0.0)
    wps = psP.tile([128, 512], F32, tag="p5")
    NWARM = 14
    for k in range(NWARM):
        nc.tensor.matmul(wps[:, 0:128], wmat[:], wmat[:],
                         start=(k == 0), stop=(k == NWARM - 1))

    prep(0)
    # emission order = scheduler priority: critical chain (sobel -> squares
    # -> rinv -> uv -> suv) first, then drains (t12/C/store), then prefetch
    for i in range(N_TILES + 2):
        if i < N_TILES:
            front_mm(i)
            front_elem(i)
        if 1 <= i <= N_TILES:
            mid(i - 1)
        if 2 <= i:
            back(i - 2)
        if i < N_TILES:
            if i + 1 < N_TILES:
                prep(i + 1)
            if i + 2 < N_TILES:
                load_x(i + 2)
            if i + 1 <= 3:
                c0, c1 = (i + 1) * NF_MATS * 128, (i + 2) * NF_MATS * 128
                nc.sync.dma_start(matsf_sb[:, c0:c1],
                                  matsf_d[:, c0:c1].bitcast(F32R))


_CACHE = {}


def _build():
    if "nc" in _CACHE:
        return _CACHE["nc"]
    nc = bacc.Bacc("TRN2", target_bir_lowering=False, debug=False)
    x_d = nc.dram_tensor("x", [ROWS, W], F32R, kind="ExternalInput").ap()
    o_d = nc.dram_tensor("O", [ROWS, 3, W], F16, kind="ExternalOutput").ap()
    import ml_dtypes
    arrf, arrb = _mats_const()
    matsf_d = nc.inline_tensor(arrf, name="matsf").ap()
    matsb_d = nc.inline_tensor(arrb.astype(ml_dtypes.bfloat16),
                               name="matsb").ap()
    # register a tiny-bias const AP for the rsqrt zero-guard
    _c = nc.alloc_sbuf_tensor("const-float32-1e-35", [128, 1], F32)
    nc.gpsimd.memset(_c.ap(), 1e-35)
    nc.const_aps.aps[(F32, 1e-35)] = _c.ap()
    with tile.TileContext(nc) as tc:
        with ExitStack() as ctx:
            _emit(ctx, tc, x_d, o_d, matsf_d, matsb_d)
    nc.compile()
    _CACHE["nc"] = nc
    return nc


def _run(inputs, trace=False):
    x = np.asarray(inputs["ivc_img"], np.float32)
    assert x.shape == (N_CORES * IPC, 1, H, W), x.shape
    nc = _build()
    in_maps = [
        {"x": np.ascontiguousarray(x[IPC * c:IPC * (c + 1), 0].reshape(ROWS, W))}
        for c in range(N_CORES)
    ]
    res = bass_utils.run_bass_kernel_spmd(
        nc, in_maps, core_ids=list(range(N_CORES)), trace=trace
    )
    outs = []
    for c in range(N_CORES):
        o = np.asarray(res.results[c]["O"], dtype=np.float32)
        o = o.reshape(IPC, H, 3, W).transpose(0, 2, 1, 3)
        outs.append(o)
    full = np.ascontiguousarray(np.concatenate(outs, axis=0))
    return full, res


def kernel(**inputs):
    full, _ = _run(inputs, trace=False)
    return full


def kernel_traced(**inputs):
    full, res = _run(inputs, trace=True)
    return full, res


# revision 6
# speedup vs baseline: 1.0330x; 1.0091x over previous
"""Trainium2 Bass kernel: NoiseEstimation (Sobel magnitude G, orientation
coherence C, 5x5 local variance V) over (16,1,512,512) fp32 input.

Math (trig-free):
  gx, gy = Sobel 3x3 cross-correlation, zero pad 1
  g2 = gx^2 + gy^2 (bf16);  r = abs_rsqrt(g2);  G = g2 * r
  u = gx*r, v = gy*r  (cos/sin of theta, bf16)
  C = u*box3c(u) + v*box3c(v)      box3c = (3x3 sum - center)/8, replicate
                                   pad; the -1/8 folds in via u^2+v^2=1
  V = box5(x^2)/25 - (box5(x)/25)^2     box5 = 5x5 sum, zero pad

Distribution: pure data parallel, 2 images per core on 8 NeuronCores.
Per core the 2 images are stacked into a [1024, 512] row space processed
as 9 overlapping 128-row tiles, software-pipelined across engines.

Engine assignment (balanced under the v2 timeline cost model; GPSIMD
cannot touch PSUM on hardware and its add/mult runs at 0.42 efficiency):
  Tensor : 18 matmuls/tile: sobel (5, fp32r, gy first), p5 = box5 of x
           via 5 shifted fp32r matmuls on a fp32 B5 band (replaces the
           hx scan - PE had headroom, DVE was the bottleneck), q5 from
           the hxx scan (bf16 B5), -I @ mu^2 folded into the open q5
           group (V = q5 - mu^2), su/sv box3 (6, bf16, shifted rhs)
  Act    : gxyb = bf16 copy of gxy PSUM (one pass; downstream squares /
           g2 / uv become bf16-2x DVE ops and the PSUM frees a slot
           early), rinv = Abs_rsqrt(g2), m2 = Square(p5), V = Copy(q5)
           -> fp16 (Copy/Square/Abs_rsqrt share one act table set)
  DVE    : hxx = 5-window running sum scan of x^2, pq2 = gxyb*gxyb,
           g2 = p2+q2, uv = gxyb*rinv (all bf16 2x), uv edge copies,
           t12 = uv*suv (PSUM mixed - only DVE can), C = t1+t2 -> fp16
  Pool   : xx = x^2 (SBUF fp32 in), G = g2*rinv -> fp16, x pad memsets
  SP/Act : stores ride SP, input prefetches ride the Act DGE queue (no
           waits -> no head-of-line blocking); the LAST store issues on
           the Act queue - the final SP-queue DMA is not reliably
           drained at program end (silent tail corruption otherwise)

Output is staged in fp16 (halves store traffic vs fp32; host upcasts,
well inside the 2e-2 tolerance). PSUM: gxy[2]x2 double-buffered, p5[1],
q5[1], suv[2] = 8 banks. PE warmup burst rides out the cold p-states.
Drain-specific tweaks: the last tile's suv borrows a free psA-ring
buffer (dodges the single-psS wait behind t12(N-2)) and its uv/t12 run
per-channel so the two channel sub-chains pipeline through the drain.

TimelineSim (v2 cost model): ~48.9us/core vs 87.2us for the original
Pool-heavy kernel (1.78x); CoreSim (v1): ~46.3us. The su/sv matmuls are
grouped so consecutive matmuls share lhsT (PE skips redundant weight
loads on hardware). Verified on the 8
axon trn2 cores: rel err 5.75e-3 (gate 2e-2). Steady state is paced by
the PE at ~3.8us/tile (18 matmuls); pipeline fill (~5us) + drain (~7us)
are the remaining overhead.
"""

import numpy as np
from contextlib import ExitStack

import concourse.bass as bass
import concourse.bacc as bacc
import concourse.tile as tile
import concourse.mybir as mybir
from concourse import bass_utils

F32 = mybir.dt.float32
F32R = mybir.dt.float32r
F16 = mybir.dt.float16
BF16 = mybir.dt.bfloat16
AL = mybir.AluOpType
AF = mybir.ActivationFunctionType

H = 512
W = 512
N_CORES = 8
IPC = 2                    # images per core
ROWS = IPC * H             # 1024 stacked rows per core
TILE_OFS = [0, 124, 248, 372, 496, 620, 744, 868, 896]
N_TILES = len(TILE_OFS)
# tile -> band-matrix set (0=t0 top, 1=interior, 2=image boundary, 3=bottom)
TILE_SET = [0, 1, 1, 1, 2, 1, 1, 1, 3]
SET_OFS = [0, 124, 496, 896]
N_SETS = 4
NF_MATS = 5   # fp32 per set: V121p, V121n, Vd1, Vd2, B5f
NB_MATS = 3   # bf16 per set: B3, B3c, B5


def _valid_range(ti):
    """Valid output partition range [m0, m1) for tile ti."""
    if ti == 0:
        return 0, 126
    if ti == N_TILES - 1:
        start_g = TILE_OFS[ti - 1] + 126   # first row not covered by prev tile
        return start_g - TILE_OFS[ti], 128
    return 2, 126


def _build_mat_set(ofs):
    """Banded 128x128 lhsT matrices for a tile at row offset `ofs`.

    lhsT[k, m] = weight of tile input row k contributing to output row m.
    Image edges (zero pad for sobel/box5, replicate for box3) are encoded
    per-row; tiles spanning the two-image boundary get block-diagonal bands.
    """
    fmats = [np.zeros((128, 128), np.float32) for _ in range(NF_MATS)]
    bmats = [np.zeros((128, 128), np.float32) for _ in range(NB_MATS)]
    V121p, V121n, Vd1, Vd2, B5f = fmats
    B3, B3c, B5 = bmats
    for m in range(128):
        g = ofs + m
        img = g // H
        if img >= IPC:
            continue
        lo, hi = img * H, img * H + H - 1

        def add(mat, d, w, rep=False):
            gt = g + d
            if gt < lo or gt > hi:
                if not rep:
                    return
                gt = min(max(gt, lo), hi)
            k = gt - ofs
            if 0 <= k < 128:
                mat[k, m] += w

        for d, wgt in ((-1, 1.0), (0, 2.0), (1, 1.0)):
            add(V121p, d, wgt)
            add(V121n, d, -wgt)
        for d, wgt in ((-1, -1.0), (1, 1.0)):
            add(Vd1, d, wgt)
            add(Vd2, d, 2.0 * wgt)
        for d in (-2, -1, 0, 1, 2):
            add(B5, d, 0.04)
            add(B5f, d, 0.04)
        for d in (-1, 0, 1):
            add(B3, d, 0.125, rep=True)
            add(B3c, d, 0.125, rep=True)
        B3c[m, m] -= 0.125  # folds C's "- 1/8" via u^2 + v^2 = 1
    return fmats, bmats


def _mats_const():
    """fp32 sobel mats [128, 4*4*128]; bf16 mats [128, (4*3+1)*128]
    (B3, B3c, B5 per set + trailing -I)."""
    arrf = np.zeros((128, N_SETS * NF_MATS * 128), np.float32)
    arrb = np.zeros((128, (N_SETS * NB_MATS + 1) * 128), np.float32)
    for s, ofs in enumerate(SET_OFS):
        fmats, bmats = _build_mat_set(ofs)
        for f, mat in enumerate(fmats):
            b = s * NF_MATS + f
            arrf[:, b * 128:(b + 1) * 128] = mat
        for f, mat in enumerate(bmats):
            b = s * NB_MATS + f
            arrb[:, b * 128:(b + 1) * 128] = mat
    arrb[:, N_SETS * NB_MATS * 128:] = -np.eye(128, dtype=np.float32)
    return arrf, arrb


def _emit(ctx: ExitStack, tc: "tile.TileContext", x_d, o_d, matsf_d, matsb_d):
    nc = tc.nc
    mpool = ctx.enter_context(tc.tile_pool(name="mats", bufs=1))
    xpool = ctx.enter_context(tc.tile_pool(name="xp", bufs=5))
    spool = ctx.enter_context(tc.tile_pool(name="sp", bufs=3))
    upool = ctx.enter_context(tc.tile_pool(name="up", bufs=3))
    opool = ctx.enter_context(tc.tile_pool(name="op", bufs=4))
    psA = ctx.enter_context(tc.tile_pool(name="psA", bufs=2, space="PSUM"))
    psP = ctx.enter_context(tc.tile_pool(name="psP", bufs=1, space="PSUM"))
    psQ = ctx.enter_context(tc.tile_pool(name="psQ", bufs=1, space="PSUM"))
    psS = ctx.enter_context(tc.tile_pool(name="psS", bufs=1, space="PSUM"))

    matsf_sb = mpool.tile([128, N_SETS * NF_MATS * 128], F32R, tag="matsf")
    matsb_sb = mpool.tile([128, (N_SETS * NB_MATS + 1) * 128], BF16, tag="matsb")

    def MF(s, f):
        b = s * NF_MATS + f
        return matsf_sb[:, b * 128:(b + 1) * 128]

    def MB(s, f):
        b = s * NB_MATS + f
        return matsb_sb[:, b * 128:(b + 1) * 128]

    In_m = matsb_sb[:, N_SETS * NB_MATS * 128:]

    # per-tile state carried across the software pipeline
    st = [None] * N_TILES
    xq = [None] * N_TILES
    sq = [None] * N_TILES

    def load_x(i):
        # x tile: [4 zero | 512 | 4 zero] cols; issued two slots ahead.
        # The pad columns are memset once per physical buffer (first 4
        # tiles): the DMA only ever writes cols 4:516, so they stay zero.
        x_t = xpool.tile([128, 520], F32R, tag="x")
        nc.scalar.dma_start(x_t[:, 4:516], x_d[TILE_OFS[i]:TILE_OFS[i] + 128, :])
        nc.gpsimd.memset(x_t[:, 0:4].bitcast(F32), 0.0)
        nc.gpsimd.memset(x_t[:, 516:520].bitcast(F32), 0.0)
        xq[i] = x_t

    def prep(i):
        # xx = x^2 (Act, bf16; zero pads square to zero), then the two
        # horizontal 5-window running sums as DVE scans:
        # state = (d0[t] + state) - d1[t]  ->  out[t] = sum x[t+1..t+5]
        x_t = xq[i]
        xf = x_t.bitcast(F32)
        xx_t = spool.tile([128, 518], BF16, tag="xx")
        nc.gpsimd.tensor_mul(xx_t[:], xf[:, 0:518], xf[:, 0:518])
        hxx_t = spool.tile([128, 513], BF16, tag="hxx")
        nc.vector.tensor_tensor_scan(hxx_t[:], xx_t[:, 5:518], xx_t[:, 0:513],
                                     xx_t[:, 4:5], AL.add, AL.subtract)
        sq[i] = hxx_t

    def front_mm(i):
        # PE-only: no upstream deps beyond x/hx/hxx -> issue first per slot.
        # gy first: its Square can overlap the gx matmuls, shortening the
        # rinv critical chain.
        s = TILE_SET[i]
        x_t = xq[i]

        def xr(j):
            return x_t[:, 4 + j:4 + j + 512]

        # ---- sobel via banded matmuls -> gxy PSUM [128, 2, 512] ----
        gxy = psA.tile([128, 2, 512], F32, tag="gxy")
        nc.tensor.matmul(gxy[:, 1, :], MF(s, 2), xr(-1), start=True, stop=False)
        nc.tensor.matmul(gxy[:, 1, :], MF(s, 2), xr(+1), start=False, stop=False)
        nc.tensor.matmul(gxy[:, 1, :], MF(s, 3), xr(0), start=False, stop=True)
        nc.tensor.matmul(gxy[:, 0, :], MF(s, 0), xr(+1), start=True, stop=False)
        nc.tensor.matmul(gxy[:, 0, :], MF(s, 1), xr(-1), start=False, stop=True)

        hxx_t = sq[i]

        # ---- box5 sums: p5 directly from x via 5 shifted fp32r matmuls
        # (kills the hx scan on the DVE); q5 from the hxx scan ----
        p5 = psP.tile([128, 512], F32, tag="p5")
        for jj, j in enumerate((-2, -1, 0, 1, 2)):
            nc.tensor.matmul(p5[:], MF(s, 4), xr(j), start=(jj == 0),
                             stop=(jj == 4))
        q5 = psQ.tile([128, 512], F32, tag="q5")
        nc.tensor.matmul(q5[:], MB(s, 2), hxx_t[:, 1:513], start=True, stop=False)

        st[i] = dict(s=s, gxy=gxy, p5=p5, q5=q5)

    def front_elem(i):
        # G-chain head + V; rinv lands at slot end, its consumers (uv, G)
        # run next slot so the spill is absorbed.
        d = st[i]
        # one Act pass converts gxy PSUM -> bf16 SBUF; squares, g2 and uv
        # then run as cheap bf16-2x DVE ops, and the PSUM frees a slot
        # earlier (no write-after-read loop through uv)
        gxyb = spool.tile([128, 2, 512], BF16, tag="gxyb")
        nc.scalar.activation(gxyb[:], d["gxy"][:], AF.Copy)
        pq2 = spool.tile([128, 2, 512], BF16, tag="pq2")
        nc.vector.tensor_mul(pq2[:], gxyb[:], gxyb[:])
        g2 = spool.tile([128, 512], BF16, tag="g2")
        nc.vector.tensor_add(g2[:], pq2[:, 0, :], pq2[:, 1, :])
        m2 = spool.tile([128, 512], BF16, tag="m2")
        nc.scalar.activation(m2[:], d["p5"][:], AF.Square)
        rinv = spool.tile([128, 512], BF16, tag="rinv")
        nc.scalar.activation(rinv[:], g2[:], AF.Abs_reciprocal_sqrt, bias=1e-35)

        # V = q5 - mu^2: -I matmul folds m2 into the open q5 group, then
        # a single Act copy moves PSUM -> fp16 SBUF
        nc.tensor.matmul(d["q5"][:], In_m, m2[:], start=False, stop=True)
        gcv = opool.tile([128, 3, 512], F16, tag="gcv")
        nc.scalar.activation(gcv[:, 2, :], d["q5"][:], AF.Copy)
        d.update(g2=g2, rinv=rinv, gcv=gcv, gxyb=gxyb)

    def mid(i):
        # one slot after front: normalize + box3 matmuls + G
        d = st[i]
        s, g2, rinv, gcv = d["s"], d["g2"], d["rinv"], d["gcv"]
        if i == N_TILES - 1:
            # last tile: per-channel in SEPARATE tiles so the u-chain
            # (suv-o0, t12-u) is not dep-coupled to the v writes during
            # the pipeline drain
            uv0 = upool.tile([128, 514], BF16, tag="uv80")
            uv1 = upool.tile([128, 514], BF16, tag="uv81")
            uv = [uv0, uv1]
            for o in range(2):
                nc.vector.tensor_mul(uv[o][:, 1:513], d["gxyb"][:, o, :],
                                     rinv[:])
                nc.vector.tensor_copy(uv[o][:, 0:1], uv[o][:, 1:2])
                nc.vector.tensor_copy(uv[o][:, 513:514], uv[o][:, 512:513])
        else:
            uv = upool.tile([128, 2, 514], BF16, tag="uv")
            rb = rinv[:].rearrange('p (o f) -> p o f', o=1).broadcast_to(
                [128, 2, 512])
            nc.vector.tensor_mul(uv[:, :, 1:513], d["gxyb"][:], rb)
            nc.vector.tensor_copy(uv[:, :, 0:1], uv[:, :, 1:2])
            nc.vector.tensor_copy(uv[:, :, 513:514], uv[:, :, 512:513])

        # G = g2 * rinv -> fp16 (Pool, SBUF only)
        nc.gpsimd.tensor_mul(gcv[:, 0, :], g2[:], rinv[:])

        # ---- su|sv: box3 matmuls on the unit gradients (2D slices; the
        # matmul ISA rejects multi-free-dim operands) ----
        # the LAST tile's suv borrows a psA-ring buffer (its gxy PSUM is
        # already consumed) so the drain never waits on the single psS
        # buffer behind t12(N-2)'s read
        if i == N_TILES - 1:
            suv = psA.tile([128, 2, 512], F32, tag="gxy")
        else:
            suv = psS.tile([128, 2, 512], F32, tag="suv")
        # same-matrix matmuls adjacent (B3c pair, then four B3 taps): the
        # PE skips redundant weight loads between consecutive matmuls that
        # share lhsT on real hardware (cost-model neutral)
        U = [uv[o] if i == N_TILES - 1 else uv[:, o, :] for o in range(2)]
        for o in range(2):
            nc.tensor.matmul(suv[:, o, :], MB(s, 1), U[o][:, 1:513],
                             start=True, stop=False)
        for o in range(2):
            nc.tensor.matmul(suv[:, o, :], MB(s, 0), U[o][:, 0:512],
                             start=False, stop=False)
        for o in range(2):
            nc.tensor.matmul(suv[:, o, :], MB(s, 0), U[o][:, 2:514],
                             start=False, stop=True)
        d.update(uv=uv, suv=suv)

    T12_SPLIT = 384   # cols 0:384 on DVE, 384:512 on Pool

    def back(j):
        # separate tiles for the two column halves: a shared tile would
        # create a false WAW between the DVE and Pool writers
        d = st[j]
        uv, suv, gcv = d["uv"], d["suv"], d["gcv"]
        t12 = upool.tile([128, 2, 512], BF16, tag="t12")
        if j == N_TILES - 1:
            nc.vector.tensor_mul(t12[:, 0, :], uv[0][:, 1:513], suv[:, 0, :])
            nc.vector.tensor_mul(t12[:, 1, :], uv[1][:, 1:513], suv[:, 1, :])
        else:
            nc.vector.tensor_mul(t12[:], uv[:, :, 1:513], suv[:])
        # C = t1 + t2 -> fp16 (DVE bf16 2x)
        nc.vector.tensor_add(gcv[:, 1, :], t12[:, 0, :], t12[:, 1, :])

        ofs = TILE_OFS[j]
        m0, m1 = _valid_range(j)
        g0 = ofs + m0
        # stores on SP (loads ride the Act queue so prefetches never queue
        # behind stores); the LAST store goes on the Act queue — the final
        # SP-queue DMA is not reliably drained at program end
        eng = nc.scalar if j == N_TILES - 1 else nc.sync
        eng.dma_start(o_d[g0:g0 + (m1 - m0), :, :], gcv[m0:m1, :, :])
        st[j] = None

    load_x(0)
    nc.sync.dma_start(matsf_sb[:, 0:NF_MATS * 128],
                      matsf_d[:, 0:NF_MATS * 128].bitcast(F32R))
    load_x(1)
    nc.sync.dma_start(matsb_sb[:], matsb_d[:])

    # force the single act table set (abs_rsqrt / square / copy) - emitted
    # AFTER the loads so the x0 DMA dispatch is not stuck behind it on the
    # Act SEQ
    scratch = mpool.tile([128, 8], F32, tag="scr")
    nc.gpsimd.memset(scratch[:], 1.0)
    nc.scalar.activation(scratch[:, 0:4], scratch[:, 4:8],
                         AF.Abs_reciprocal_sqrt, bias=1e-35)

    # PE warmup: the PE clock ramps only while busy (HAM gate); a burst of
    # dummy N=128 matmuls on zeroed weights rides out the cold p-states
    # during the initial DMA window so tile 0 runs at full rate.
    wmat = mpool.tile([128, 128], BF16, tag="wm")
    nc.gpsimd.memset(wmat[:], 0.0)
    wps = psP.tile([128, 512], F32, tag="p5")
    NWARM = 14
    for k in range(NWARM):
        nc.tensor.matmul(wps[:, 0:128], wmat[:], wmat[:],
                         start=(k == 0), stop=(k == NWARM - 1))

    prep(0)
    # emission order = scheduler priority: critical chain (sobel -> squares
    # -> rinv -> uv -> suv) first, then drains (t12/C/store), then prefetch
    for i in range(N_TILES + 2):
        if i < N_TILES:
            front_mm(i)
            front_elem(i)
        if 1 <= i <= N_TILES:
            mid(i - 1)
        if 2 <= i:
            back(i - 2)
        if i < N_TILES:
            if i + 1 < N_TILES:
                prep(i + 1)
            if i + 2 < N_TILES:
                load_x(i + 2)
            if i + 1 <= 3:
                c0, c1 = (i + 1) * NF_MATS * 128, (i + 2) * NF_MATS * 128
                nc.sync.dma_start(matsf_sb[:, c0:c1],
                                  matsf_d[:, c0:c1].bitcast(F32R))


_CACHE = {}


def _build():
    if "nc" in _CACHE:
        return _CACHE["nc"]
    nc = bacc.Bacc("TRN2", target_bir_lowering=False, debug=False)
    x_d = nc.dram_tensor("x", [ROWS, W], F32R, kind="ExternalInput").ap()
    o_d = nc.dram_tensor("O", [ROWS, 3, W], F16, kind="ExternalOutput").ap()
    import ml_dtypes
    arrf, arrb = _mats_const()
    matsf_d = nc.inline_tensor(arrf, name="matsf").ap()
    matsb_d = nc.inline_tensor(arrb.astype(ml_dtypes.bfloat16),
                               name="matsb").ap()
    # register a tiny-bias const AP for the rsqrt zero-guard
    _c = nc.alloc_sbuf_tensor("const-float32-1e-35", [128, 1], F32)
    nc.gpsimd.memset(_c.ap(), 1e-35)
    nc.const_aps.aps[(F32, 1e-35)] = _c.ap()
    with tile.TileContext(nc) as tc:
        with ExitStack() as ctx:
            _emit(ctx, tc, x_d, o_d, matsf_d, matsb_d)
    nc.compile()
    _CACHE["nc"] = nc
    return nc


def _run(inputs, trace=False):
    x = np.asarray(inputs["ivc_img"], np.float32)
    assert x.shape == (N_CORES * IPC, 1, H, W), x.shape
    nc = _build()
    in_maps = [
        {"x": np.ascontiguousarray(x[IPC * c:IPC * (c + 1), 0].reshape(ROWS, W))}
        for c in range(N_CORES)
    ]
    res = bass_utils.run_bass_kernel_spmd(
        nc, in_maps, core_ids=list(range(N_CORES)), trace=trace
    )
    outs = []
    for c in range(N_CORES):
        o = np.asarray(res.results[c]["O"], dtype=np.float32)
        o = o.reshape(IPC, H, 3, W).transpose(0, 2, 1, 3)
        outs.append(o)
    full = np.ascontiguousarray(np.concatenate(outs, axis=0))
    return full, res


def kernel(**inputs):
    full, _ = _run(inputs, trace=False)
    return full


def kernel_traced(**inputs):
    full, res = _run(inputs, trace=True)
    return full, res
